# revision 1
# baseline (speedup 1.0000x reference)
"""Causal self-attention nn module (B=4, T=2048, E=1024, H=16, HS=64) on 8
TRN2 cores — faithful to the reference's raw .view() reshape [b,t,h,hs] ->
[h,b,t,hs].

That reshape makes the attention run over 64 independent "sequences": each
sequence is one 128-timestep block of one batch, with its 16 heads
interleaved into 2048 positions (t2 = tau*16 + h).  Sequence (b, s') covers
x[b, 128*s' : 128*(s'+1), :], and its attention output lands back in rows
[128*s', 128*(s'+1)) of att_cat[b] — so sharding by sequence blocks needs no
cross-core reduction at all.

Sharding: core c handles batch b = c//2, rows t in [1024*(c%2), +1024) — 8
sequences.  Each core computes full rows of the output; host concatenates
and adds proj_b.

Per-core kernel (matmuls in fp32r):
  A: q/k in interleaved-transposed layout Qseq/Kseq [2 seqs x 64hs, 2048 t2]
     per sequence pair; v via a DRAM round-trip into V tiles [128 t2, 65]
     (col 64 = ones -> softmax denominators for free).
  B: causal flash attention per sequence in t2 space: S.T = K @ Q.T with two
     sequences row-packed in the PE array, exp on ACT, triangular-diagonal
     masking only (band tiles use narrowed matmuls + zero memsets),
     P.T @ V_aug accumulated in PSUM, normalize via DVE reciprocal +
     gpsimd partition_broadcast, written strided into att_cat.T layout.
  C: y rows = att_cat @ proj_w.T  (full projection, no reduction).
"""

import numpy as np
from contextlib import ExitStack

import concourse.bass as bass
import concourse.mybir as mybir
import concourse.tile as tile
from concourse import bacc
from concourse.bass_utils import run_bass_kernel_spmd

F32 = mybir.dt.float32
F32R = mybir.dt.float32r
U32 = mybir.dt.uint32
AF = mybir.ActivationFunctionType

B, T, E, H, HS = 4, 2048, 1024, 16, 64
NCORES = 8
TCORE = T * B // NCORES      # 1024 rows per core
DH = H * HS                  # 1024
NEG = -1.0e9
ONE_BITS = int(np.float32(1.0).view(np.uint32))
SCALE = HS ** -0.5

# band tile geometry: for diagonal-band tile jj, scores only needed in
# columns [128*jj, 512); matmul N must be >=256 for full-rate fp32r.
BAND_C0 = [0, 128, 256, 256]     # first column the matmul writes
BAND_A0 = [0, 128, 256, 256]     # first column the @v matmul writes


def build_nc(t_core=TCORE, e=E, e_out=E):
    assert t_core % 512 == 0
    nseq = t_core // 128     # sequences (= tau tiles)
    nsp = nseq // 2          # sequence pairs
    ne = e // 128
    nhp = H // 2             # 8 head pairs
    ntk = 16                 # t2 tiles per sequence (2048/128)
    niq = 4                  # t2 query blocks (2048/512)
    neo = e_out // 512

    nc = bacc.Bacc("TRN2", debug=False, num_devices=1)

    xT_d = nc.dram_tensor("xT", [e, t_core], F32R, kind="ExternalInput")
    wq_d = nc.dram_tensor("wq", [e, DH], F32R, kind="ExternalInput")
    wk_d = nc.dram_tensor("wk", [e, DH], F32R, kind="ExternalInput")
    wv_d = nc.dram_tensor("wv", [e, DH], F32R, kind="ExternalInput")
    pw_d = nc.dram_tensor("pwT", [DH, e_out], F32R, kind="ExternalInput")
    tri_d = nc.dram_tensor("tri", [128, 128], F32, kind="ExternalInput")
    y_d = nc.dram_tensor("y", [t_core, e_out], F32, kind="ExternalOutput")

    with tile.TileContext(nc) as tc, ExitStack() as ctx:
        p_keep = ctx.enter_context(tc.tile_pool(name="keep", bufs=1))
        Qseq = p_keep.tile([128, nsp, 2048], F32R, tag="Qseq")
        Kseq = p_keep.tile([128, nsp, 2048], F32R, tag="Kseq")
        tri_sb = p_keep.tile([128, 128], F32, tag="tri")
        nc.sync.dma_start(out=tri_sb, in_=tri_d.ap())

        # per (pair, tk) stationary [128 t2, 192]: cols 0:64 = V of seq A,
        # 64:128 = ones (replicates the softmax denominator across 64 PSUM
        # partitions in the @v matmul), 128:192 = V of seq B
        p_vsb = ctx.enter_context(tc.tile_pool(name="vsb", bufs=1))
        v_sb = p_vsb.tile([128, nsp, ntk, 192], F32R, tag="v")
        nc.vector.memset(v_sb.bitcast(U32), ONE_BITS)

        p_drm = ctx.enter_context(tc.tile_pool(name="drm", bufs=1, space="DRAM"))
        vscr = p_drm.tile([t_core, DH], F32R, tag="vscr")

        # ---------------- phase A ----------------
        with tc.tile_pool(name="px", bufs=1) as p_x, tc.tile_pool(
            name="Aps", bufs=6, space="PSUM"
        ) as p_Aps:
            xT = p_x.tile([128, ne, t_core], F32R, tag="xT")
            for ei in range(ne):
                nc.sync.dma_start(
                    out=xT[:, ei, :],
                    in_=xT_d.ap().rearrange("(a p) t -> p a t", p=128)[:, ei, :],
                )

            # v first -> DRAM scratch, then gather into V tiles (t2 on
            # partitions); the round-trip latency overlaps the q/k matmuls
            with tc.tile_pool(name="pwv", bufs=1) as p_wv, tc.tile_pool(
                name="vstg", bufs=3
            ) as p_stg:
                wv_sb = p_wv.tile([128, ne, DH], F32R, tag="wv")
                wv_r = wv_d.ap().rearrange("(a p) d -> p a d", p=128)
                for ei in range(ne):
                    nc.sync.dma_start(out=wv_sb[:, ei, :], in_=wv_r[:, ei, :])
                for tt in range(nseq):
                    for c2 in range(DH // 512):
                        ps = p_Aps.tile([128, 512], F32, tag="Aps", name="Aps")
                        for ei in range(ne):
                            nc.tensor.matmul(
                                ps,
                                xT[:, ei, bass.ts(tt, 128)],
                                wv_sb[:, ei, bass.ts(c2, 512)],
                                start=(ei == 0),
                                stop=(ei == ne - 1),
                            )
                        stg = p_stg.tile([128, 512], F32R, tag="stg")
                        nc.vector.tensor_copy(stg, ps)
                        nc.sync.dma_start(
                            out=vscr[bass.ts(tt, 128), bass.ts(c2, 512)],
                            in_=stg,
                        )
                for j in range(nseq):
                    for tk in range(ntk):
                        src = vscr[
                            j * 128 + 8 * tk : j * 128 + 8 * tk + 8, :
                        ].rearrange("a (h c) -> (a h) c", c=64)
                        c0 = 128 * (j % 2)
                        nc.sync.dma_start(
                            out=v_sb[:, j // 2, tk, c0 : c0 + 64],
                            in_=src,
                        )

            # q/k -> interleaved transposed sequence layout (one weight
            # tensor resident at a time to fit SBUF)
            for name, dram, dst in (("wq", wq_d, Qseq), ("wk", wk_d, Kseq)):
                with tc.tile_pool(name=f"p_{name}", bufs=1) as p_w:
                    w_sb = p_w.tile([128, ne, DH], F32R, tag=name, name=name)
                    w_r = dram.ap().rearrange("(a p) d -> p a d", p=128)
                    for ei in range(ne):
                        nc.sync.dma_start(
                            out=w_sb[:, ei, :], in_=w_r[:, ei, :]
                        )
                    for hp in range(nhp):
                        for c in range(t_core // 512):
                            ps = p_Aps.tile(
                                [128, 512], F32, tag="Aps", name="Aps"
                            )
                            for ei in range(ne):
                                nc.tensor.matmul(
                                    ps,
                                    w_sb[:, ei, hp * 128 : hp * 128 + 128],
                                    xT[:, ei, bass.ts(c, 512)],
                                    start=(ei == 0),
                                    stop=(ei == ne - 1),
                                )
                            # scatter into dst: seq j = 4c+j4, head h=2hp+hh,
                            # col t2 = tau*16 + h
                            for hh in range(2):
                                h = 2 * hp + hh
                                pv = ps[64 * hh : 64 * hh + 64, :].rearrange(
                                    "p (j4 tau) -> p j4 tau", j4=4
                                )
                                for par in range(2):
                                    # j4 = par, par+2 -> same partition block
                                    dv = dst[
                                        64 * par : 64 * par + 64, :, :
                                    ].rearrange(
                                        "p sp (tau hx) -> p sp tau hx", hx=16
                                    )
                                    nc.vector.tensor_copy(
                                        dv[:, 2 * c : 2 * c + 2, :, h],
                                        pv[:, par::2, :],
                                    )


        # ---------------- phase B: attention ----------------
        p_big = ctx.enter_context(tc.tile_pool(name="big", bufs=1))
        attC = p_big.tile([128, 8, t_core], F32R, tag="attC")
        pwT = p_big.tile([128, 8, e_out], F32R, tag="pwT")
        nc.sync.dma_start(
            out=pwT, in_=pw_d.ap().rearrange("(g p) E -> p g E", p=128)
        )

        with tc.tile_pool(name="attn", bufs=6) as p_at, tc.tile_pool(
            name="nrm", bufs=3
        ) as p_nrm, tc.tile_pool(
            name="st_ps", bufs=4, space="PSUM"
        ) as p_st, tc.tile_pool(
            name="av_ps", bufs=4, space="PSUM"
        ) as p_av:
            # normalization of group (sp, iq) is emitted while the NEXT
            # group's matmuls run, so the in-order ACT queue never stalls
            # the next group's exp chain (which would bubble the PE and
            # keep HAM re-throttling it)
            pending_norm = None
            for sp in range(nsp):
                for iq in range(niq):
                    n_tk = 4 * (iq + 1)
                    av = {}
                    for hh in range(2):
                        av[hh] = p_av.tile([128, 512], F32, tag="av", name="av")
                    for tk in range(n_tk):
                        jj = tk - 4 * iq  # >=0 -> diagonal band tile
                        st = {}
                        for hh in range(2):
                            st[hh] = p_st.tile(
                                [128, 512], F32, tag="st", name="st"
                            )
                            sl = slice(64 * hh, 64 * hh + 64)
                            c0 = BAND_C0[jj] if jj >= 0 else 0
                            nc.tensor.matmul(
                                st[hh][:, c0:512],
                                Kseq[sl, sp, bass.ts(tk, 128)],
                                Qseq[sl, sp, iq * 512 + c0 : (iq + 1) * 512],
                                start=True,
                                stop=True,
                            )
                        for hh in range(2):
                            pt = p_at.tile([128, 512], F32R, tag="pt", name="pt")
                            if jj < 0:
                                nc.scalar.activation(
                                    pt, st[hh], AF.Exp, scale=SCALE
                                )
                                a0 = 0
                            else:
                                d0 = 128 * jj
                                if d0 > 0:
                                    nc.vector.memset(
                                        pt[:, 0:d0].bitcast(U32), 0
                                    )
                                sm = p_nrm.tile(
                                    [128, 128], F32, tag="sm", name="sm"
                                )
                                nc.vector.tensor_add(
                                    sm, st[hh][:, d0 : d0 + 128], tri_sb
                                )
                                nc.scalar.activation(
                                    pt[:, d0 : d0 + 128], sm, AF.Exp,
                                    scale=SCALE,
                                )
                                if d0 + 128 < 512:
                                    nc.scalar.activation(
                                        pt[:, d0 + 128 : 512],
                                        st[hh][:, d0 + 128 : 512],
                                        AF.Exp,
                                        scale=SCALE,
                                    )
                                a0 = BAND_A0[jj]
                            nc.tensor.matmul(
                                av[hh][:, a0:512],
                                v_sb[:, sp, tk, 64 * hh : 64 * hh + 128],
                                pt[:, a0:512],
                                start=(tk == 0),
                                stop=(tk == n_tk - 1),
                                skip_group_check=True,
                            )
                    # av rows: hh=0 -> out 0:64, denominator 64:128 (ones
                    # block replicated it); hh=1 -> denominator 0:64, out
                    # 64:128.  Copy out rows into att_cat.T layout (DVE),
                    # build 1/denominator as exp(-ln d) on ACT (DVE divide
                    # is unsupported and DVE reciprocal is 6.5ns/elem), then
                    # scale with one DVE multiply.
                    # early part reads av (PSUM) so its banks release fast:
                    # unnormalized copies into attC (DVE) + ln(denominator)
                    # on ACT.  The rest (exp(-ln) + in-place scaling) defers
                    # one group so it never blocks the next group's exps in
                    # the in-order ACT queue.
                    lnts = {}
                    for hh in range(2):
                        j = 2 * sp + hh
                        o0 = 64 * hh
                        d0 = 64 - o0
                        lnt = p_nrm.tile(
                            [128, 512], F32, tag="lnt", bufs=3, name="lnt"
                        )
                        lnts[hh] = lnt
                        nc.scalar.activation(
                            lnt[0:64, :], av[hh][d0 : d0 + 64, :], AF.Ln
                        )
                        nc.scalar.activation(
                            lnt[64:128, :], av[hh][d0 : d0 + 64, :], AF.Ln
                        )
                        avv = av[hh][o0 : o0 + 64, :].rearrange(
                            "p (tau g r) -> p r g tau", tau=32, g=8, r=2
                        )
                        for r in range(2):
                            nc.vector.tensor_copy(
                                attC[
                                    64 * r : 64 * r + 64,
                                    :,
                                    j * 128 + iq * 32 : j * 128 + iq * 32 + 32,
                                ],
                                avv[:, r, :, :],
                            )

                    def emit_norm(sp=sp, iq=iq, lnts=lnts):
                        for hh in range(2):
                            j = 2 * sp + hh
                            rcb = p_nrm.tile(
                                [128, 512], F32, tag="rcb", bufs=3, name="rcb"
                            )
                            nc.scalar.activation(
                                rcb, lnts[hh], AF.Exp, scale=-1.0
                            )
                            rcbv = rcb.rearrange(
                                "p (tau g r) -> p r g tau", tau=32, g=8, r=2
                            )
                            for r in range(2):
                                sl = attC[
                                    64 * r : 64 * r + 64,
                                    :,
                                    j * 128 + iq * 32 : j * 128 + iq * 32 + 32,
                                ]
                                nc.vector.tensor_mul(
                                    sl, sl, rcbv[64 * r : 64 * r + 64, r, :, :]
                                )

                    if pending_norm is not None:
                        pending_norm()
                    pending_norm = emit_norm
            if pending_norm is not None:
                pending_norm()

        # ---------------- phase C: projection ----------------
        with tc.tile_pool(name="out", bufs=3) as p_out, tc.tile_pool(
            name="Cps", bufs=4, space="PSUM"
        ) as p_Cps:
            for tt in range(nseq):
                y_sb = p_out.tile([128, e_out], F32, tag="y")
                for eh in range(neo):
                    ps = p_Cps.tile([128, 512], F32, tag="Cps", name="Cps")
                    for g in range(8):
                        nc.tensor.matmul(
                            ps,
                            attC[:, g, bass.ts(tt, 128)],
                            pwT[:, g, bass.ts(eh, 512)],
                            start=(g == 0),
                            stop=(g == 7),
                        )
                    nc.vector.tensor_copy(y_sb[:, bass.ts(eh, 512)], ps)
                nc.sync.dma_start(out=y_d.ap()[bass.ts(tt, 128), :], in_=y_sb)

    nc.compile()
    return nc


def make_tri():
    x = np.arange(128, dtype=np.int32)[:, None]
    y = np.arange(128, dtype=np.int32)[None, :]
    return np.where(y - x >= 0, 0.0, NEG).astype(np.float32)


def shard_inputs(x, Wq, Wk, Wv, proj_w):
    wqF = np.ascontiguousarray(np.transpose(Wq, (1, 0, 2)).reshape(E, DH))
    wkF = np.ascontiguousarray(np.transpose(Wk, (1, 0, 2)).reshape(E, DH))
    wvF = np.ascontiguousarray(np.transpose(Wv, (1, 0, 2)).reshape(E, DH))
    pwTf = np.ascontiguousarray(proj_w.T)
    tri = make_tri()
    in_maps = []
    for c in range(NCORES):
        b = c // 2
        t0 = TCORE * (c % 2)
        in_maps.append(
            {
                "xT": np.ascontiguousarray(x[b, t0 : t0 + TCORE, :].T),
                "wq": wqF,
                "wk": wkF,
                "wv": wvF,
                "pwT": pwTf,
                "tri": tri,
            }
        )
    return in_maps


_cached_nc = None


def get_nc():
    global _cached_nc
    if _cached_nc is None:
        _cached_nc = build_nc()
    return _cached_nc


def kernel(x, Wq, Wk, Wv, proj_w, proj_b, _trace=False, _tmpdir=None):
    x = np.asarray(x, dtype=np.float32)
    Wq = np.asarray(Wq, dtype=np.float32)
    Wk = np.asarray(Wk, dtype=np.float32)
    Wv = np.asarray(Wv, dtype=np.float32)
    proj_w = np.asarray(proj_w, dtype=np.float32)
    proj_b = np.asarray(proj_b, dtype=np.float32)

    nc = get_nc()
    in_maps = shard_inputs(x, Wq, Wk, Wv, proj_w)
    res = run_bass_kernel_spmd(nc, in_maps, core_ids=list(range(NCORES)))

    out = np.empty((B, T, E), dtype=np.float32)
    for c in range(NCORES):
        b = c // 2
        t0 = TCORE * (c % 2)
        out[b, t0 : t0 + TCORE] = res.results[c]["y"] + proj_b
    return out



# revision 15
# speedup vs baseline: 1.4698x; 1.4698x over previous
"""Causal self-attention nn module (B=4, T=2048, E=1024, H=16, HS=64) on 8
TRN2 cores — faithful to the reference's raw .view() reshape [b,t,h,hs] ->
[h,b,t,hs].

That reshape makes the attention run over 64 independent "sequences": each
sequence is one 128-timestep block of one batch, with its 16 heads
interleaved into 2048 positions (t2 = tau*16 + h).  Sequence (b, s') covers
x[b, 128*s' : 128*(s'+1), :], and its attention output lands back in rows
[128*s', 128*(s'+1)) of att_cat[b] — so sharding by sequence blocks needs no
cross-core reduction at all.

Sharding: core c handles batch b = c//2, rows t in [1024*(c%2), +1024) — 8
sequences.  Each core computes full rows of the output; host concatenates
and adds proj_b.

v2 design notes (all matmul operands bf16, host-cast):
  A: all three weight tensors resident in SBUF; v first (DRAM round-trip
     for the [t,h*hs]->[t2,hs] partition reshape overlaps the q/k matmuls),
     then q/k into interleaved-transposed Qseq/Kseq bf16.
  B: flash attention per sequence pair; scores into [128,1024] 2-bank PSUM
     pair tiles so one exp covers two key tiles (band tile paired with a
     full tile -> contiguous exp range); causal diag masked by a 0/1 bf16
     multiply on DVE (no tri add / split exps on ACT); softmax denominator
     via the ones-column trick; normalization uses DVE
     reciprocal_approx_fast + fused multiply into attC (ACT does exp ONLY).
     av matmuls are emitted two score-pairs behind so the PE queue's sem
     waits are pre-satisfied (keeps the LDW/MM pipeline dense and HAM warm).
  C: projection emitted per sequence-pair right after its normalization, so
     it hides under phase B's ACT work.
"""

import numpy as np
from collections import deque
from contextlib import ExitStack

import ml_dtypes

import concourse.bass as bass
import concourse.mybir as mybir
import concourse.tile as tile
from concourse import bacc
from concourse.bass_utils import run_bass_kernel_spmd

F32 = mybir.dt.float32
BF16 = mybir.dt.bfloat16
U32 = mybir.dt.uint32
AF = mybir.ActivationFunctionType

B, T, E, H, HS = 4, 2048, 1024, 16, 64
NCORES = 8
TCORE = T * B // NCORES      # 1024 rows per core
DH = H * HS                  # 1024
ONE2_BITS = 0x3F803F80       # two bf16 1.0s
SCALE = HS ** -0.5


def build_nc(t_core=TCORE, e=E, e_out=E, debug_taps=False):
    assert t_core % 512 == 0
    nseq = t_core // 128     # sequences (= tau tiles)
    nsp = nseq // 2          # sequence pairs
    ne = e // 128
    nhp = H // 2             # 8 head pairs
    ntk = 16                 # t2 tiles per sequence (2048/128)
    niq = 4                  # t2 query blocks (2048/512)
    neo = e_out // 512

    nc = bacc.Bacc("TRN2", debug=False, num_devices=1)

    xT_d = nc.dram_tensor("xT", [e, t_core], BF16, kind="ExternalInput")
    wq_d = nc.dram_tensor("wq", [e, DH], BF16, kind="ExternalInput")
    wk_d = nc.dram_tensor("wk", [e, DH], BF16, kind="ExternalInput")
    wv_d = nc.dram_tensor("wv", [e, DH], BF16, kind="ExternalInput")
    pw_d = nc.dram_tensor("pwT", [DH, e_out], BF16, kind="ExternalInput")
    tri_d = nc.dram_tensor("tri", [128, 128], BF16, kind="ExternalInput")
    y_d = nc.dram_tensor("y", [t_core, e_out], F32, kind="ExternalOutput")
    if debug_taps:
        dbg_pt_d = nc.dram_tensor("dbg_pt", [128, 1024], BF16, kind="ExternalOutput")
        dbg_av_d = nc.dram_tensor("dbg_av", [128, 512], F32, kind="ExternalOutput")
        dbg_rcb_d = nc.dram_tensor("dbg_rcb", [128, 512], F32, kind="ExternalOutput")
        dbg_rcb2_d = nc.dram_tensor("dbg_rcb2", [64, 512], F32, kind="ExternalOutput")
        dbg_attc_d = nc.dram_tensor(
            "dbg_attc", [128, 8, 256], BF16, kind="ExternalOutput"
        )

    with tile.TileContext(nc) as tc, ExitStack() as ctx:
        p_keep = ctx.enter_context(tc.tile_pool(name="keep", bufs=1))
        xT = p_keep.tile([128, ne, t_core], BF16, tag="xT")
        # weight slabs: [q | k | v] along the middle dim
        w_all = p_keep.tile([128, 3 * ne, DH], BF16, tag="w_all")
        Qseq = p_keep.tile([128, nsp, 2048], BF16, tag="Qseq")
        Kseq = p_keep.tile([128, nsp, 2048], BF16, tag="Kseq")
        tri_sb = p_keep.tile([128, 128], BF16, tag="tri")
        pwT = p_keep.tile([128, 8, e_out], BF16, tag="pwT")
        attC = [
            p_keep.tile([128, 8, 256], BF16, tag=f"attC{sp}", name=f"attC{sp}")
            for sp in range(nsp)
        ]

        # per (pair, tk) stationary [128 t2, 192]: cols 0:64 = V of seq A,
        # 64:128 = ones (replicates the softmax denominator across 64 PSUM
        # partitions in the @v matmul), 128:192 = V of seq B
        v_sb = p_keep.tile([128, nsp, ntk, 192], BF16, tag="v")
        nc.vector.memset(v_sb.bitcast(U32), ONE2_BITS)

        nc.sync.dma_start(out=tri_sb, in_=tri_d.ap())
        nc.sync.dma_start(
            out=xT, in_=xT_d.ap().rearrange("(a p) t -> p a t", p=128)
        )
        for wi, wd in enumerate((wq_d, wk_d, wv_d)):
            nc.sync.dma_start(
                out=w_all[:, wi * ne : (wi + 1) * ne, :],
                in_=wd.ap().rearrange("(a p) d -> p a d", p=128),
            )
        nc.sync.dma_start(
            out=pwT, in_=pw_d.ap().rearrange("(g p) E -> p g E", p=128)
        )

        p_drm = ctx.enter_context(tc.tile_pool(name="drm", bufs=1, space="DRAM"))
        vscr = p_drm.tile([t_core, DH], BF16, tag="vscr")

        # ---------------- phase A ----------------
        with tc.tile_pool(name="Aps", bufs=4, space="PSUM") as p_Aps, \
             tc.tile_pool(name="vstg", bufs=3) as p_stg:
            # v first -> DRAM scratch, then gather into V tiles (t2 on
            # partitions); the round-trip latency overlaps the q/k matmuls
            for tt in range(nseq):
                for c2 in range(DH // 512):
                    ps = p_Aps.tile([128, 512], F32, tag="Aps", name="Aps")
                    for ei in range(ne):
                        nc.tensor.matmul(
                            ps,
                            xT[:, ei, bass.ts(tt, 128)],
                            w_all[:, 2 * ne + ei, bass.ts(c2, 512)],
                            start=(ei == 0),
                            stop=(ei == ne - 1),
                        )
                    stg = p_stg.tile([128, 512], BF16, tag="stg")
                    nc.vector.tensor_copy(stg, ps)
                    nc.sync.dma_start(
                        out=vscr[bass.ts(tt, 128), bass.ts(c2, 512)],
                        in_=stg,
                    )
            for j in range(nseq):
                for tk in range(ntk):
                    src = vscr[
                        j * 128 + 8 * tk : j * 128 + 8 * tk + 8, :
                    ].rearrange("a (h c) -> (a h) c", c=64)
                    c0 = 128 * (j % 2)
                    nc.sync.dma_start(
                        out=v_sb[:, j // 2, tk, c0 : c0 + 64],
                        in_=src,
                    )

            # q/k -> interleaved transposed sequence layout
            for wi, dst in ((0, Qseq), (1, Kseq)):
                for hp in range(nhp):
                    for c in range(t_core // 512):
                        ps = p_Aps.tile([128, 512], F32, tag="Aps", name="Aps")
                        for ei in range(ne):
                            nc.tensor.matmul(
                                ps,
                                w_all[:, wi * ne + ei, hp * 128 : hp * 128 + 128],
                                xT[:, ei, bass.ts(c, 512)],
                                start=(ei == 0),
                                stop=(ei == ne - 1),
                            )
                        # scatter into dst: seq j = 4c+j4, head h=2hp+hh,
                        # col t2 = tau*16 + h
                        for hh in range(2):
                            h = 2 * hp + hh
                            pv = ps[64 * hh : 64 * hh + 64, :].rearrange(
                                "p (j4 tau) -> p j4 tau", j4=4
                            )
                            for par in range(2):
                                # j4 = par, par+2 -> same partition block
                                dv = dst[
                                    64 * par : 64 * par + 64, :, :
                                ].rearrange(
                                    "p sp (tau hx) -> p sp tau hx", hx=16
                                )
                                nc.vector.tensor_copy(
                                    dv[:, 2 * c : 2 * c + 2, :, h],
                                    pv[:, par::2, :],
                                )

        # ---------------- phase B: attention (+ interleaved projection) ----
        with tc.tile_pool(name="pt", bufs=6) as p_pt, tc.tile_pool(
            name="nrm", bufs=4
        ) as p_nrm, tc.tile_pool(
            name="st_ps", bufs=2, space="PSUM"
        ) as p_st, tc.tile_pool(
            name="av_ps", bufs=2, space="PSUM"
        ) as p_av, tc.tile_pool(
            name="Cps", bufs=2, space="PSUM"
        ) as p_Cps, tc.tile_pool(
            name="out", bufs=2
        ) as p_out:
            for sp in range(nsp):
                for iq in range(niq):
                    n_tk = 4 * (iq + 1)
                    av = {}
                    for hh in range(2):
                        av[hh] = p_av.tile([128, 512], F32, tag="av", name="av")
                    # pair list: (tileA, tileB); band tile jj is paired with
                    # a full tile so ONE exp covers [128*jj : 1024]
                    # contiguously.  iq=0 has no full tiles: band-band pairs
                    # with two exp slices.
                    fulls = list(range(4 * iq))
                    bands = [4 * iq + jj for jj in range(4)]
                    pairs = []
                    if iq == 0:
                        pairs = [(bands[0], bands[1]), (bands[2], bands[3])]
                    else:
                        for jj in range(4):
                            pairs.append((bands[jj], fulls[jj]))
                        rest = fulls[4:]
                        for i in range(0, len(rest), 2):
                            pairs.append((rest[i], rest[i + 1]))

                    first_av = {0: True, 1: True}
                    n_av_emitted = {0: 0, 1: 0}
                    pending = deque()

                    def emit_pair(sp, iq, pair, av, first_av, n_av_emitted):
                        tkA, tkB = pair
                        jjA = tkA - 4 * iq
                        jjB = tkB - 4 * iq
                        c0A = 128 * jjA if jjA >= 0 else 0
                        c0B = 128 * jjB if jjB >= 0 else 0
                        pts = {}
                        for hh in range(2):
                            sl = slice(64 * hh, 64 * hh + 64)
                            stp = p_st.tile([128, 1024], F32, tag="st", name="st")
                            nc.tensor.matmul(
                                stp[:, c0A:512],
                                Kseq[sl, sp, bass.ts(tkA, 128)],
                                Qseq[sl, sp, iq * 512 + c0A : (iq + 1) * 512],
                                start=True,
                                stop=True,
                            )
                            nc.tensor.matmul(
                                stp[:, 512 + c0B : 1024],
                                Kseq[sl, sp, bass.ts(tkB, 128)],
                                Qseq[sl, sp, iq * 512 + c0B : (iq + 1) * 512],
                                start=True,
                                stop=True,
                            )
                            pt = p_pt.tile([128, 1024], BF16, tag="pt", name="pt")
                            pts[hh] = pt
                            if jjB < 0:
                                # (band|full) or (full|full): one contiguous exp
                                nc.scalar.activation(
                                    pt[:, c0A:1024], stp[:, c0A:1024],
                                    AF.Exp, scale=SCALE,
                                )
                            else:
                                # band-band (iq==0): two exp slices
                                nc.scalar.activation(
                                    pt[:, c0A:512], stp[:, c0A:512],
                                    AF.Exp, scale=SCALE,
                                )
                                nc.scalar.activation(
                                    pt[:, 512 + c0B : 1024],
                                    stp[:, 512 + c0B : 1024],
                                    AF.Exp, scale=SCALE,
                                )
                            # causal diag mask: multiply by 0/1 tri
                            if jjA >= 0:
                                nc.vector.tensor_mul(
                                    pt[:, c0A : c0A + 128],
                                    pt[:, c0A : c0A + 128],
                                    tri_sb,
                                )
                            if jjB >= 0:
                                nc.vector.tensor_mul(
                                    pt[:, 512 + c0B : 512 + c0B + 128],
                                    pt[:, 512 + c0B : 512 + c0B + 128],
                                    tri_sb,
                                )
                            if (
                                debug_taps and sp == 0 and iq == 1
                                and pair == (4, 0) and hh == 0
                            ):
                                nc.sync.dma_start(out=dbg_pt_d.ap(), in_=pt)

                        def emit_avs():
                            for hh in range(2):
                                pt = pts[hh]
                                for half, tk, c0 in (
                                    (0, tkA, c0A), (1, tkB, c0B)
                                ):
                                    n_av_emitted[hh] += 1
                                    nc.tensor.matmul(
                                        av[hh][:, c0:512],
                                        v_sb[:, sp, tk, 64 * hh : 64 * hh + 128],
                                        pt[:, 512 * half + c0 : 512 * half + 512],
                                        start=first_av[hh],
                                        stop=(n_av_emitted[hh] == n_tk),
                                        skip_group_check=True,
                                    )
                                    first_av[hh] = False

                        return emit_avs

                    for pair in pairs:
                        pending.append(
                            emit_pair(sp, iq, pair, av, first_av, n_av_emitted)
                        )
                        if len(pending) > 2:
                            pending.popleft()()
                    while pending:
                        pending.popleft()()

                    if debug_taps and sp == 0 and iq == 1:
                        avc = p_nrm.tile([128, 512], F32, tag="avc", name="avc")
                        nc.vector.tensor_copy(avc, av[0])
                        nc.sync.dma_start(out=dbg_av_d.ap(), in_=avc)

                    # normalization: rcb = 1/denominator on DVE (not ACT),
                    # replicated to both partition halves so the in-place
                    # multiply stays base-partition aligned (2-input DVE ops
                    # need matching bases; 1-input copies may shift base).
                    # av rows: hh=0 -> out 0:64, denominator 64:128; hh=1 ->
                    # denominator 0:64, out 64:128 (ones block replicated it
                    # across 64 partitions).
                    for hh in range(2):
                        o0 = 64 * hh
                        d0 = 64 - o0
                        # custom-DVE bit-trick ops misread PSUM on HW (sim
                        # models it fine) — stage the denominator into SBUF.
                        denS = p_nrm.tile([64, 512], F32, tag="denS", name="denS")
                        nc.vector.tensor_copy(denS, av[hh][d0 : d0 + 64, :])
                        rcb = p_nrm.tile([128, 512], F32, tag="rcb", name="rcb")
                        nc.vector.reciprocal_approx_fast(rcb[0:64, :], denS)
                        nc.vector.tensor_copy(rcb[64:128, :], rcb[0:64, :])
                        if debug_taps and sp == 0 and iq == 1 and hh == 0:
                            nc.sync.dma_start(out=dbg_rcb_d.ap(), in_=rcb)
                        avv = av[hh][o0 : o0 + 64, :].rearrange(
                            "p (tau g r) -> p r g tau", tau=32, g=8, r=2
                        )
                        rcbv = rcb.rearrange(
                            "p (tau g r) -> p r g tau", tau=32, g=8, r=2
                        )
                        for r in range(2):
                            sl = attC[sp][
                                64 * r : 64 * r + 64,
                                :,
                                hh * 128 + iq * 32 : hh * 128 + iq * 32 + 32,
                            ]
                            nc.vector.tensor_copy(sl, avv[:, r, :, :])
                            nc.vector.tensor_mul(
                                sl, sl, rcbv[64 * r : 64 * r + 64, r, :, :]
                            )

                if debug_taps and sp == 0:
                    nc.sync.dma_start(out=dbg_attc_d.ap(), in_=attC[0])

                # ---- projection for this sequence pair (hides under ACT) --
                for tt2 in range(2):
                    tt = 2 * sp + tt2
                    y_sb = p_out.tile([128, e_out], F32, tag="y")
                    for eh in range(neo):
                        ps = p_Cps.tile([128, 512], F32, tag="Cps", name="Cps")
                        for g in range(8):
                            nc.tensor.matmul(
                                ps,
                                attC[sp][:, g, tt2 * 128 : tt2 * 128 + 128],
                                pwT[:, g, bass.ts(eh, 512)],
                                start=(g == 0),
                                stop=(g == 7),
                            )
                        nc.vector.tensor_copy(y_sb[:, bass.ts(eh, 512)], ps)
                    nc.sync.dma_start(
                        out=y_d.ap()[bass.ts(tt, 128), :], in_=y_sb
                    )

    nc.compile()
    return nc


def make_tri01():
    x = np.arange(128, dtype=np.int32)[:, None]
    y = np.arange(128, dtype=np.int32)[None, :]
    return np.where(y - x >= 0, 1.0, 0.0).astype(ml_dtypes.bfloat16)


def shard_inputs(x, Wq, Wk, Wv, proj_w):
    bf = ml_dtypes.bfloat16
    wqF = np.ascontiguousarray(
        np.transpose(Wq, (1, 0, 2)).reshape(E, DH)
    ).astype(bf)
    wkF = np.ascontiguousarray(
        np.transpose(Wk, (1, 0, 2)).reshape(E, DH)
    ).astype(bf)
    wvF = np.ascontiguousarray(
        np.transpose(Wv, (1, 0, 2)).reshape(E, DH)
    ).astype(bf)
    pwTf = np.ascontiguousarray(proj_w.T).astype(bf)
    tri = make_tri01()
    in_maps = []
    for c in range(NCORES):
        b = c // 2
        t0 = TCORE * (c % 2)
        in_maps.append(
            {
                "xT": np.ascontiguousarray(
                    x[b, t0 : t0 + TCORE, :].T
                ).astype(bf),
                "wq": wqF,
                "wk": wkF,
                "wv": wvF,
                "pwT": pwTf,
                "tri": tri,
            }
        )
    return in_maps


_cached_nc = None


def get_nc():
    global _cached_nc
    if _cached_nc is None:
        _cached_nc = build_nc()
    return _cached_nc


def kernel(x, Wq, Wk, Wv, proj_w, proj_b, _trace=False, _tmpdir=None):
    x = np.asarray(x, dtype=np.float32)
    Wq = np.asarray(Wq, dtype=np.float32)
    Wk = np.asarray(Wk, dtype=np.float32)
    Wv = np.asarray(Wv, dtype=np.float32)
    proj_w = np.asarray(proj_w, dtype=np.float32)
    proj_b = np.asarray(proj_b, dtype=np.float32)

    nc = get_nc()
    in_maps = shard_inputs(x, Wq, Wk, Wv, proj_w)
    res = run_bass_kernel_spmd(nc, in_maps, core_ids=list(range(NCORES)))

    out = np.empty((B, T, E), dtype=np.float32)
    for c in range(NCORES):
        b = c // 2
        t0 = TCORE * (c % 2)
        out[b, t0 : t0 + TCORE] = res.results[c]["y"] + proj_b
    return out
